# revision 1
# baseline (speedup 1.0000x reference)
"""DiffuseRouter kernel for 8 TRN2 NeuronCores.

Reference computation (enable_time=False, soft_time_routing=True):
    out[b, l, d] = (1/3) * sum_g sum_e expert_emb_g[e, b, l, d]
i.e. a uniform-weighted sum of 28 expert planes per batch element.

Sharding: pure data-parallel over batch B=8 -> one batch element per core.
Each core reads its 28 [256, 1280] f32 planes (36.7 MB), reduces them
on-chip, scales by 1/3, and writes its [256, 1280] output.  No collectives
needed (B == n_cores).

v8 = v3 + paired-plane loads.  SDMA engines run 381 ns per 10,240 B
descriptor (26.9 GB/s) vs a ~29 GB/s line rate -> ~28 ns fixed cost per
descriptor.  Planes 0..25 are host-packed in PAIRS so each partition's
bytes for two planes are contiguous: one [128, 5120] transfer per pair
with 20,480 B descriptors halves the per-descriptor overhead on the
86 us load stream.  Plane 26 loads alone, plane 27 in column chunks for
a short tail.

Compute (unchanged from v3): TensorE sums columns [0, 1536) of each
plane via identity matmuls into 3 PSUM banks (fp32r, 1 cycle/row; PE
stays at the 1.2 GHz MID clock for fp32r so 3 banks is the budget);
ACT scales x1/3 out of PSUM.  DVE sums columns [1536, 2560) with a
scalar_tensor_tensor chain, scale folded in.  Only natural full-128-
partition contiguous transfers run the engines at line rate; partial/
repacked shapes measured 20-30% slower, so engine 15's intermittent
~22 GB/s mode is accepted rather than dodged.
"""

import numpy as np

import concourse.bacc as bacc
import concourse.tile as tile
from concourse import mybir
from concourse.alu_op_type import AluOpType
from concourse.bass_utils import run_bass_kernel_spmd

N_CORES = 8
E_TOTAL = 28  # 4 + 8 + 16 experts across the 3 granularity levels
L, D = 256, 1280
P = 128  # SBUF partitions
FD = (L // P) * D  # 2560 free-dim elements per partition
BW = 512  # one 2 KB PSUM bank of f32
NB_PE = 3  # banks summed on TensorE (cols 0..1536)
DVE_LO = NB_PE * BW  # 1536: start of the DVE column range
DVE_W = FD - DVE_LO  # 1024 cols summed on DVE
SCALE = 1.0 / 3.0
N_PAIR = 13  # planes 0..25 load as pairs; 26 alone; 27 chunked

_NC_CACHE = None


def _build_nc():
    """Build the SPMD Bass program (identical on all 8 cores)."""
    nc = bacc.Bacc(
        "TRN2", target_bir_lowering=False, debug=False, enable_partition_id=False
    )
    f32 = mybir.dt.float32
    f32r = mybir.dt.float32r
    # Pairs: xp[g, p, :] = [plane_2g rows(2p,2p+1) | plane_2g+1 rows(2p,2p+1)]
    xp = nc.dram_tensor("xp", [N_PAIR, P, 2 * FD], f32, kind="ExternalInput")
    x26 = nc.dram_tensor("x26", [P, FD], f32, kind="ExternalInput")
    x27 = nc.dram_tensor("x27", [P, FD], f32, kind="ExternalInput")
    ident_d = nc.dram_tensor("ident", [P, P], f32, kind="ExternalInput")
    out = nc.dram_tensor("out", [L, D], f32, kind="ExternalOutput")

    xp_r = xp.ap().bitcast(f32r)
    x26_r = x26.ap().bitcast(f32r)
    x27_a = x27.ap()
    x27_r = x27_a.bitcast(f32r)
    out_t = out.ap().rearrange("(p a) d -> p (a d)", a=2)

    mult = AluOpType.mult
    add = AluOpType.add

    with tile.TileContext(nc) as tc:
        with (
            tc.tile_pool(name="in", bufs=5) as pin,
            tc.tile_pool(name="one", bufs=1) as pone,
            tc.tile_pool(name="const", bufs=1) as pconst,
            tc.tile_pool(name="acc", bufs=1) as pacc,
            tc.tile_pool(name="ps", bufs=1, space="PSUM") as pps,
        ):
            ident = pconst.tile([P, P], f32r, name="ident", tag="ident")
            # Identity comes in on the ACT ring so the SP ring carries
            # nothing but the plane loads.
            nc.scalar.dma_start(out=ident[:], in_=ident_d.ap().bitcast(f32r))
            psums = [
                pps.tile([P, BW], f32, name=f"ps{b}", tag=f"ps{b}")
                for b in range(NB_PE)
            ]
            outs = pacc.tile([P, NB_PE * BW], f32, name="outs", tag="outs")
            acc = pacc.tile([P, DVE_W], f32, name="acc", tag="acc")

            last = E_TOTAL - 1

            def consume(e, pe_chunks, dve_chunks):
                """Issue this plane's PE matmuls / DVE adds (+ finals)."""
                for b in range(NB_PE):
                    nc.tensor.matmul(
                        psums[b][:],
                        ident[:],
                        pe_chunks[b],
                        start=(e == 0),
                        stop=(e == last),
                    )
                    if e == last:
                        bs = slice(b * BW, (b + 1) * BW)
                        nc.scalar.mul(outs[:, bs], psums[b][:], SCALE)
                        nc.scalar.dma_start(out=out_t[:, bs], in_=outs[:, bs])
                for lo, w, th in dve_chunks:
                    qs = slice(lo - DVE_LO, lo - DVE_LO + w)
                    if e == 0:
                        nc.vector.tensor_scalar_mul(acc[:, qs], th, SCALE)
                    else:
                        nc.vector.scalar_tensor_tensor(
                            acc[:, qs], th, SCALE, acc[:, qs], mult, add
                        )
                    if e == last:
                        # DVE-range stores ride the sync ring (idle after the
                        # last chunk-load trigger) so they don't queue behind
                        # the PE banks' COPY/store pairs on the ACT sequencer.
                        nc.sync.dma_start(
                            out=out_t[:, lo : lo + w], in_=acc[:, qs]
                        )

            # Pairs: one [128, 5120] load covers planes 2g and 2g+1; their
            # column layouts sit side by side in the tile.
            for g in range(N_PAIR):
                t = pin.tile([P, 2 * FD], f32r)
                nc.sync.dma_start(out=t[:], in_=xp_r[g])
                for half in range(2):
                    o = half * FD
                    consume(
                        2 * g + half,
                        [t[:, o + b * BW : o + (b + 1) * BW] for b in range(NB_PE)],
                        [(DVE_LO, DVE_W, t[:, o + DVE_LO : o + FD].bitcast(f32))],
                    )

            # Plane 26: single natural load.
            t26 = pone.tile([P, FD], f32r, name="t26", tag="t26")
            nc.sync.dma_start(out=t26[:], in_=x26_r)
            consume(
                26,
                [t26[:, b * BW : (b + 1) * BW] for b in range(NB_PE)],
                [(DVE_LO, DVE_W, t26[:, DVE_LO:FD].bitcast(f32))],
            )

            # Plane 27: column chunks; PE's first, small DVE chunk last.
            pe_chunks = []
            for b in range(NB_PE):
                ct = pone.tile([P, BW], f32r, name=f"c{b}", tag=f"c{b}")
                nc.sync.dma_start(out=ct[:], in_=x27_r[:, b * BW : (b + 1) * BW])
                pe_chunks.append(ct[:])
            dve_chunks = []
            for lo, w in ((DVE_LO, 768), (DVE_LO + 768, 256)):
                ct = pone.tile([P, w], f32, name=f"d{lo}", tag=f"d{lo}")
                nc.sync.dma_start(out=ct[:], in_=x27_a[:, lo : lo + w])
                dve_chunks.append((lo, w, ct[:]))
            consume(27, pe_chunks, dve_chunks)
    nc.compile()
    return nc


def _get_nc():
    global _NC_CACHE
    if _NC_CACHE is None:
        _NC_CACHE = _build_nc()
    return _NC_CACHE


def _run(inputs, trace=False, trace_kwargs=None):
    e0 = np.asarray(inputs["expert_emb_0"], dtype=np.float32)
    e1 = np.asarray(inputs["expert_emb_1"], dtype=np.float32)
    e2 = np.asarray(inputs["expert_emb_2"], dtype=np.float32)
    B = e0.shape[1]
    assert B == N_CORES, f"expected B == {N_CORES}, got {B}"

    ident = np.eye(P, dtype=np.float32)
    in_maps = []
    for b in range(B):
        xb = np.concatenate([e0[:, b], e1[:, b], e2[:, b]], axis=0)
        # [28, 256, 1280] -> per-plane partition-major [28, 128, 2560]
        v = xb.reshape(E_TOTAL, P, FD)
        # pairs [13, 128, 5120]: partition p holds plane 2g then 2g+1
        pairs = (
            v[: 2 * N_PAIR]
            .reshape(N_PAIR, 2, P, FD)
            .transpose(0, 2, 1, 3)
            .reshape(N_PAIR, P, 2 * FD)
        )
        in_maps.append(
            {
                "xp": np.ascontiguousarray(pairs),
                "x26": np.ascontiguousarray(v[26]),
                "x27": np.ascontiguousarray(v[27]),
                "ident": ident,
            }
        )

    kw = {}
    if trace:
        kw["trace"] = True
        if trace_kwargs:
            kw.update(trace_kwargs)
    try:
        res = run_bass_kernel_spmd(_get_nc(), in_maps, list(range(N_CORES)), **kw)
    except Exception:
        # One retry: transient device errors usually clear on re-dispatch.
        res = run_bass_kernel_spmd(_get_nc(), in_maps, list(range(N_CORES)), **kw)
    out = np.stack([res.results[b]["out"] for b in range(B)], axis=0)
    return out.astype(np.float32, copy=False), res


def kernel(**inputs) -> np.ndarray:
    out, _ = _run(inputs, trace=False)
    return out

